# revision 1
# baseline (speedup 1.0000x reference)
"""Trainium2 Bass kernel for nn_BLCD_Loss (retrieval_knn).

Math: for l2-normalized rows, ||a-b||^2 = 2 - 2*a.b, so all pairwise
distances come from two small Gram matmuls per core. The top-(K+1)
neighbor selection reduces to a per-row threshold (17th largest cosine)
found with two rounds of the DVE 8-wide `max` + `match_replace` ops, and
the neighbor gather becomes a 0/1 mask multiply. Self-pairs are excluded
up-front by subtracting a large constant on the (local) diagonal.

Sharding: 256 anchor rows -> 32 rows on each of 8 cores; each core gets
the full yi^T (256KB) plus its local slices, computes a partial scalar
loss; the host sums the 8 partials.
"""

import numpy as np

N, D, K = 256, 256, 16
M_MARGIN, T_THRESH, EPS = 0.6, 0.0025, 1e-12
NCORES, RPC = 8, 32  # cores, rows per core
BIG = 1000.0

_CACHE = {}


def _build():
    from concourse import bacc, mybir, tile
    import concourse.bass as bass

    dt = mybir.dt.float32
    Alu = mybir.AluOpType
    Act = mybir.ActivationFunctionType

    nc = bacc.Bacc("TRN2", target_bir_lowering=False, debug=False)

    yiT_d = nc.dram_tensor("yiT", [D, N], dt, kind="ExternalInput")
    yiLT_d = nc.dram_tensor("yiLT", [D, RPC], dt, kind="ExternalInput")
    yitT_d = nc.dram_tensor("yitT", [D, RPC], dt, kind="ExternalInput")
    ylcat_d = nc.dram_tensor("ylcat", [RPC, 2 * D], dt, kind="ExternalInput")
    eyeB_d = nc.dram_tensor("eyeB", [RPC, N], dt, kind="ExternalInput")
    i32_d = nc.dram_tensor("i32", [RPC, RPC], dt, kind="ExternalInput")
    eyeN_d = nc.dram_tensor("eyeN", [RPC, N], dt, kind="ExternalInput")
    out_d = nc.dram_tensor("out", [1, 1], dt, kind="ExternalOutput")

    with tile.TileContext(nc) as tc:
        with (
            tc.tile_pool(name="sb", bufs=1) as sb,
            tc.tile_pool(name="ps", bufs=1, space=bass.MemorySpace.PSUM) as ps,
        ):
            yiT0 = sb.tile([128, N], dt)
            yiT1 = sb.tile([128, N], dt)
            nc.sync.dma_start(yiT0[0:64, :], yiT_d[0:64, :])
            nc.sync.dma_start(yiT0[64:128, :], yiT_d[64:128, :])
            nc.sync.dma_start(yiT1[0:64, :], yiT_d[128:192, :])
            nc.sync.dma_start(yiT1[64:128, :], yiT_d[192:256, :])
            yiLT0 = sb.tile([128, RPC], dt)
            yiLT1 = sb.tile([128, RPC], dt)
            nc.sync.dma_start(yiLT0[:], yiLT_d[0:128, :])
            nc.sync.dma_start(yiLT1[:], yiLT_d[128:256, :])
            yitT0 = sb.tile([128, RPC], dt)
            yitT1 = sb.tile([128, RPC], dt)
            nc.sync.dma_start(yitT0[:], yitT_d[0:128, :])
            nc.sync.dma_start(yitT1[:], yitT_d[128:256, :])
            ylcat = sb.tile([RPC, 2 * D], dt)
            nc.sync.dma_start(ylcat[:], ylcat_d[:, :])
            eyeB = sb.tile([RPC, N], dt)
            nc.sync.dma_start(eyeB[:], eyeB_d[:, :])
            i32 = sb.tile([RPC, RPC], dt)
            nc.sync.dma_start(i32[:], i32_d[:, :])
            eyeN = sb.tile([RPC, N], dt)
            nc.sync.dma_start(eyeN[:], eyeN_d[:, :])
            ones = sb.tile([128, RPC], dt)
            nc.vector.memset(ones[:], 1.0)
            cEPS = sb.tile([128, 1], dt)
            nc.vector.memset(cEPS[:], EPS)
            cHALF = sb.tile([128, 1], dt)
            nc.vector.memset(cHALF[:], 0.5)

            # ---- column norms of yi: s_j = sum_d yi[j,d]^2 via ones-matmul
            sq0 = sb.tile([128, N], dt)
            sq1 = sb.tile([128, N], dt)
            nc.vector.tensor_tensor(sq0[:], yiT0[:], yiT0[:], op=Alu.mult)
            nc.vector.tensor_tensor(sq1[:], yiT1[:], yiT1[:], op=Alu.mult)
            ps_s = ps.tile([1, N], dt)
            nc.tensor.matmul(ps_s[:], ones[:, 0:1], sq0[:], start=True, stop=False)
            nc.tensor.matmul(ps_s[:], ones[:, 0:1], sq1[:], start=False, stop=True)
            t_row = sb.tile([1, N], dt)
            nc.scalar.activation(t_row[:], ps_s[:], Act.Sqrt, bias=cEPS[0:1, :], scale=1.0)
            inv_row = sb.tile([1, N], dt)
            nc.vector.reciprocal(inv_row[:], t_row[:])
            # broadcast inv_row down 32 partitions via K=1 matmul
            ps_b = ps.tile([RPC, N], dt)
            nc.tensor.matmul(ps_b[:], ones[0:1, 0:RPC], inv_row[:], start=True, stop=True)

            # ---- raw Gram matrices (local rows x all)
            ps_R = ps.tile([RPC, N], dt)
            nc.tensor.matmul(ps_R[:], yiLT0[:], yiT0[:], start=True, stop=False)
            nc.tensor.matmul(ps_R[:], yiLT1[:], yiT1[:], start=False, stop=False)
            nc.tensor.matmul(ps_R[:], i32[:], eyeN[:], start=False, stop=True)
            ps_Rt = ps.tile([RPC, N], dt)
            nc.tensor.matmul(ps_Rt[:], yitT0[:], yiT0[:], start=True, stop=False)
            nc.tensor.matmul(ps_Rt[:], yitT1[:], yiT1[:], start=False, stop=True)

            # ---- norms of local yi and yi_t rows in one TT+reduce pass
            scrN = sb.tile([RPC, 2 * D], dt)
            nc.vector.tensor_tensor(scrN[:], ylcat[:], ylcat[:], op=Alu.mult)
            nrm2 = sb.tile([RPC, 2], dt)
            nc.vector.tensor_reduce(
                nrm2[:], scrN[:].rearrange("p (g x) -> p g x", g=2),
                axis=mybir.AxisListType.X, op=Alu.add)
            t2 = sb.tile([RPC, 2], dt)
            nc.scalar.activation(t2[:], nrm2[:], Act.Sqrt, bias=cEPS[0:RPC, :], scale=1.0)
            inv2 = sb.tile([RPC, 2], dt)
            nc.vector.reciprocal(inv2[:], t2[:])
            sc_loc = sb.tile([RPC, 1], dt)
            nc.vector.tensor_scalar_mul(sc_loc[:], inv2[:, 0:1], -0.5)
            sc_t = sb.tile([RPC, 1], dt)
            nc.vector.tensor_scalar_mul(sc_t[:], inv2[:, 1:2], -0.5)
            sc_tB = sb.tile([RPC, 1], dt)
            nc.vector.tensor_scalar_mul(sc_tB[:], inv2[:, 1:2], -0.5 / BIG)

            # ---- column-normalized Grams (row scale folded into ACT later)
            # (compiler rejects two PSUM operands in one TensorTensor)
            b_sb = sb.tile([RPC, N], dt)
            nc.vector.tensor_copy(b_sb[:], ps_b[:])
            work = sb.tile([RPC, N], dt)
            nc.vector.tensor_tensor(work[:], ps_R[:], b_sb[:], op=Alu.mult)
            H1 = sb.tile([RPC, N], dt)
            nc.vector.tensor_tensor(H1[:], ps_Rt[:], b_sb[:], op=Alu.mult)

            # dis[i,j] = 0.5*sqrt(2-2*cos) = sqrt(-0.5*inv_i*G1 + 0.5)
            dis = sb.tile([RPC, N], dt)
            nc.scalar.activation(dis[:], work[:], Act.Sqrt, bias=cHALF[0:RPC, :], scale=sc_loc[:])
            dis_t = sb.tile([RPC, N], dt)
            nc.scalar.activation(dis_t[:], H1[:], Act.Sqrt, bias=cHALF[0:RPC, :], scale=sc_t[:])

            # ---- top-16 neighbor threshold per row (self already pushed low)
            m1 = sb.tile([RPC, 8], dt)
            nc.vector.max(out=m1[:], in_=work[:])
            w2 = sb.tile([RPC, N], dt)
            nc.vector.match_replace(
                out=w2[:], in_to_replace=m1[:], in_values=work[:], imm_value=-BIG
            )
            m2 = sb.tile([RPC, 8], dt)
            nc.vector.max(out=m2[:], in_=w2[:])
            mask = sb.tile([RPC, N], dt)
            nc.vector.tensor_scalar(
                mask[:], work[:], m2[:, 7:8], None, op0=Alu.is_ge
            )

            # ---- e1 = sum over neighbors of (dis - dis_t)^2
            diff = sb.tile([RPC, N], dt)
            nc.vector.tensor_sub(diff[:], dis[:], dis_t[:])
            mdiff = sb.tile([RPC, N], dt)
            nc.vector.tensor_tensor(mdiff[:], diff[:], mask[:], op=Alu.mult)
            scrC = sb.tile([RPC, N], dt)
            nc.vector.tensor_tensor(scrC[:], mdiff[:], mdiff[:], op=Alu.mult)
            e1row = sb.tile([RPC, 1], dt)
            nc.vector.tensor_reduce(e1row[:], scrC[:], axis=mybir.AxisListType.X, op=Alu.add)

            # ---- e2 = sum relu(dis(yi,yit) + margin - second_nn)
            scrD = sb.tile([RPC, N], dt)
            nc.vector.tensor_tensor(scrD[:], H1[:], eyeB[:], op=Alu.mult)
            hd2 = sb.tile([RPC, 1], dt)
            nc.vector.tensor_reduce(hd2[:], scrD[:], axis=mybir.AxisListType.X, op=Alu.add)
            dis_ii = sb.tile([RPC, 1], dt)
            nc.scalar.activation(dis_ii[:], hd2[:], Act.Sqrt, bias=cHALF[0:RPC, :], scale=sc_tB[:])
            dis2 = sb.tile([RPC, 1], dt)
            nc.scalar.activation(dis2[:], m1[:, 0:1], Act.Sqrt, bias=cHALF[0:RPC, :], scale=sc_loc[:])
            bias2 = sb.tile([RPC, 1], dt)
            nc.vector.tensor_scalar(
                bias2[:], dis2[:], -1.0, M_MARGIN, op0=Alu.mult, op1=Alu.add
            )
            e2row = sb.tile([RPC, 1], dt)
            nc.scalar.activation(e2row[:], dis_ii[:], Act.Relu, bias=bias2[:], scale=1.0)

            # ---- combine + partition-reduce via ones-matmul
            tot = sb.tile([RPC, 1], dt)
            nc.vector.tensor_add(tot[:], e1row[:], e2row[:])
            ps_f = ps.tile([1, 1], dt)
            nc.tensor.matmul(ps_f[:], ones[0:RPC, 0:1], tot[:], start=True, stop=True)
            outsb = sb.tile([1, 1], dt)
            nc.vector.tensor_scalar_add(outsb[:], ps_f[:], -float(RPC * K * T_THRESH))
            nc.sync.dma_start(out_d[:], outsb[:])

    nc.compile()
    return nc


def _in_maps(yi, yi_t):
    yi = np.ascontiguousarray(np.asarray(yi, np.float32))
    yi_t = np.ascontiguousarray(np.asarray(yi_t, np.float32))
    yiT = np.ascontiguousarray(yi.T)
    maps = []
    for c in range(NCORES):
        r0 = c * RPC
        eyeB = np.zeros((RPC, N), np.float32)
        eyeB[np.arange(RPC), r0 + np.arange(RPC)] = BIG
        maps.append({
            "yiT": yiT,
            "yiLT": np.ascontiguousarray(yi[r0:r0 + RPC].T),
            "yitT": np.ascontiguousarray(yi_t[r0:r0 + RPC].T),
            "ylcat": np.ascontiguousarray(
                np.hstack([yi[r0:r0 + RPC], yi_t[r0:r0 + RPC]])),
            "eyeB": eyeB,
            "i32": np.eye(RPC, dtype=np.float32),
            "eyeN": -eyeB,
        })
    return maps


def kernel(yi, yi_t):
    from concourse.bass_utils import run_bass_kernel_spmd

    if "nc" not in _CACHE:
        _CACHE["nc"] = _build()
    nc = _CACHE["nc"]
    res = run_bass_kernel_spmd(nc, _in_maps(yi, yi_t), list(range(NCORES)))
    partials = [res.results[c]["out"][0, 0] for c in range(NCORES)]
    return np.float32(np.sum(partials, dtype=np.float64))



# revision 3
# speedup vs baseline: 1.4231x; 1.4231x over previous
"""Trainium2 Bass kernel for nn_BLCD_Loss (retrieval_knn).

Math: for l2-normalized rows, ||a-b||^2 = 2 - 2*a.b, so all pairwise
distances come from small Gram matmuls per core (float32r, 4x PE rate).
Per-core inputs are column-rolled so every core's self-pair lands on the
local diagonal; self-exclusion is then a core-uniform -BIG*I accumulated
into the Gram via a third matmul whose operands are generated on-device.
Column norms come from a replicated ones-matmul (all 32 partitions get
the column sums), so no broadcast matmul or PSUM copy is needed. Top-16
selection is two rounds of DVE max8 + match_replace; the masked
sum of (dis - dis_t)^2 is one fused scalar_tensor_tensor with
accumulate.

Sharding: 256 anchor rows -> 32 rows on each of 8 cores; each core
returns [32,2] partial rows (e1, e2 terms); the host sums them.
"""

import numpy as np

N, D, K = 256, 256, 16
M_MARGIN, T_THRESH, EPS = 0.6, 0.0025, 1e-12
NCORES, RPC = 8, 32
BIG = 1.0e5

_CACHE = {}


def _build():
    from concourse import bacc, mybir, tile
    import concourse.bass as bass

    dt = mybir.dt.float32
    dtr = mybir.dt.float32r
    Alu = mybir.AluOpType
    Act = mybir.ActivationFunctionType

    nc = bacc.Bacc("TRN2", target_bir_lowering=False, debug=False)

    # pA cols: 0:32 yiLT0 | 32:64 yiLT1 | 64:96 yitT0 | 96:128 yitT1 |
    #          128:384 yiT rows 0:128 (rolled) | 384:640 yiT rows 128:256
    pA_d = nc.dram_tensor("pA", [128, 640], dt, kind="ExternalInput")
    pB_d = nc.dram_tensor("pB", [RPC, 2 * D], dt, kind="ExternalInput")
    out_d = nc.dram_tensor("out", [RPC, 2], dt, kind="ExternalOutput")

    with tile.TileContext(nc) as tc:
        with (
            tc.tile_pool(name="sb", bufs=1) as sb,
            tc.tile_pool(name="ps", bufs=1, space=bass.MemorySpace.PSUM) as ps,
        ):
            sbA = sb.tile([128, 640], dt)
            sbB = sb.tile([RPC, 2 * D], dt)

            # ---- Pool: constants (cheap, before anything else needs them)
            cEPS = sb.tile([128, 1], dt)
            nc.gpsimd.memset(cEPS[:], EPS)
            cHALF = sb.tile([128, 1], dt)
            nc.gpsimd.memset(cHALF[:], 0.5)
            zz = sb.tile([RPC, N], dt)
            nc.gpsimd.memset(zz[:], 0.0)
            eyeN = sb.tile([RPC, N], dt)
            nc.gpsimd.affine_select(
                eyeN[:], zz[:], [[1, N]], Alu.not_equal, -BIG,
                base=0, channel_multiplier=-1)
            i32 = sb.tile([RPC, RPC], dt)
            nc.gpsimd.affine_select(
                i32[:], zz[:, 0:RPC], [[1, RPC]], Alu.not_equal, 1.0,
                base=0, channel_multiplier=-1)
            # B tile DMA from the Pool queue (parallel with the others)
            nc.gpsimd.dma_start(sbB[:], pB_d[:, :])

            # ---- input DMAs on two more queues
            nc.sync.dma_start(sbA[:], pA_d[:, :])
            ones = sb.tile([128, RPC], dt)
            nc.vector.memset(ones[:], 1.0)

            yiT0 = sbA[:, 128:384]
            yiT1 = sbA[:, 384:640]
            yiLT0 = sbA[:, 0:32]
            yiLT1 = sbA[:, 32:64]
            yitT0 = sbA[:, 64:96]
            yitT1 = sbA[:, 96:128]

            # ---- Act: dummy sqrt first so the act table loads at t~0
            dummy = sb.tile([1, 1], dt)
            nc.scalar.activation(dummy[:], cEPS[0:1, :], Act.Sqrt,
                                 bias=cEPS[0:1, :], scale=1.0)

            # ---- Gram matmuls (f32r = 1 cycle/row at 256-wide output)
            ps_R = ps.tile([RPC, N], dt)
            nc.tensor.matmul(ps_R[:], yiLT0.bitcast(dtr), yiT0.bitcast(dtr),
                             start=True, stop=False)
            nc.tensor.matmul(ps_R[:], yiLT1.bitcast(dtr), yiT1.bitcast(dtr),
                             start=False, stop=False)
            nc.tensor.matmul(ps_R[:], i32[:].bitcast(dtr), eyeN[:].bitcast(dtr),
                             start=False, stop=True)

            # ---- column sums of yiT^2, replicated on 32 partitions
            sqA = sb.tile([128, 2 * D], dt)
            nc.scalar.activation(sqA[:], sbA[:, 128:640], Act.Square,
                                 bias=cEPS[:, :], scale=1.0)
            ps_s = ps.tile([RPC, N], dt)
            nc.tensor.matmul(ps_s[:], ones[:].bitcast(dtr),
                             sqA[:, 0:256].bitcast(dtr), start=True, stop=False)
            nc.tensor.matmul(ps_s[:], ones[:].bitcast(dtr),
                             sqA[:, 256:512].bitcast(dtr), start=False, stop=True)

            ps_Rt = ps.tile([RPC, N], dt)
            nc.tensor.matmul(ps_Rt[:], yitT0.bitcast(dtr), yiT0.bitcast(dtr),
                             start=True, stop=False)
            nc.tensor.matmul(ps_Rt[:], yitT1.bitcast(dtr), yiT1.bitcast(dtr),
                             start=False, stop=True)

            # ---- local row norms + cross dot from the B tile (DVE, early)
            sqB = sb.tile([RPC, 2 * D], dt)
            nc.vector.tensor_tensor(sqB[:], sbB[:], sbB[:], op=Alu.mult)
            nrm2 = sb.tile([RPC, 2], dt)
            nc.vector.tensor_reduce(
                nrm2[:], sqB[:].rearrange("p (g x) -> p g x", g=2),
                axis=mybir.AxisListType.X, op=Alu.add)
            scr0 = sb.tile([RPC, N], dt)
            dot_ii = sb.tile([RPC, 1], dt)
            nc.vector.tensor_tensor_reduce(
                scr0[:], sbB[:, 0:256], sbB[:, 256:512], 1.0, 0.0,
                op0=Alu.mult, op1=Alu.add, accum_out=dot_ii[:])

            # ---- t_b = sqrt(colsum + eps); inv_b = 1/t_b
            t_b = sb.tile([RPC, N], dt)
            nc.scalar.activation(t_b[:], ps_s[:], Act.Sqrt,
                                 bias=cEPS[0:RPC, :], scale=1.0)
            t2 = sb.tile([RPC, 2], dt)
            nc.scalar.activation(t2[:], nrm2[:], Act.Sqrt,
                                 bias=cEPS[0:RPC, :], scale=1.0)
            inv_b = sb.tile([RPC, N], dt)
            nc.vector.reciprocal(inv_b[:], t_b[:])
            inv2 = sb.tile([RPC, 2], dt)
            nc.vector.reciprocal(inv2[:], t2[:])
            sc2 = sb.tile([RPC, 2], dt)
            nc.vector.tensor_scalar_mul(sc2[:], inv2[:], -0.5)
            u = sb.tile([RPC, 1], dt)
            nc.vector.tensor_tensor(u[:], dot_ii[:], inv2[:, 0:1], op=Alu.mult)

            # ---- normalized Grams
            work = sb.tile([RPC, N], dt)
            nc.vector.tensor_tensor(work[:], ps_R[:], inv_b[:], op=Alu.mult)
            # H1 on Pool so DVE can start the max chain immediately
            H1 = sb.tile([RPC, N], dt)
            nc.gpsimd.tensor_tensor(H1[:], ps_Rt[:], inv_b[:], op=Alu.mult)

            # ---- top-16 threshold per row (self sits at -BIG on the diag)
            m1 = sb.tile([RPC, 8], dt)
            nc.vector.max(out=m1[:], in_=work[:])
            w2 = sb.tile([RPC, N], dt)
            nc.vector.match_replace(
                out=w2[:], in_to_replace=m1[:], in_values=work[:],
                imm_value=-BIG)
            m2 = sb.tile([RPC, 8], dt)
            nc.vector.max(out=m2[:], in_=w2[:])

            # ---- distances (row scale folded into activation scale)
            dis_t = sb.tile([RPC, N], dt)
            nc.scalar.activation(dis_t[:], H1[:], Act.Sqrt,
                                 bias=cHALF[0:RPC, :], scale=sc2[:, 1:2])
            dis = sb.tile([RPC, N], dt)
            nc.scalar.activation(dis[:], work[:], Act.Sqrt,
                                 bias=cHALF[0:RPC, :], scale=sc2[:, 0:1])
            dis2 = sb.tile([RPC, 1], dt)
            nc.scalar.activation(dis2[:], m1[:, 0:1], Act.Sqrt,
                                 bias=cHALF[0:RPC, :], scale=sc2[:, 0:1])
            dis_ii = sb.tile([RPC, 1], dt)
            nc.scalar.activation(dis_ii[:], u[:], Act.Sqrt,
                                 bias=cHALF[0:RPC, :], scale=sc2[:, 1:2])

            outsb = sb.tile([RPC, 2], dt)

            # ---- e2 = relu(dis_ii + (margin - dis2))
            bias2 = sb.tile([RPC, 1], dt)
            nc.vector.tensor_scalar(
                bias2[:], dis2[:], -1.0, M_MARGIN, op0=Alu.mult, op1=Alu.add)
            nc.scalar.activation(outsb[:, 1:2], dis_ii[:], Act.Relu,
                                 bias=bias2[:], scale=1.0)

            # ---- e1 = sum over neighbors of (dis - dis_t)^2, fused
            diff = sb.tile([RPC, N], dt)
            nc.vector.tensor_sub(diff[:], dis[:], dis_t[:])
            diffsq = sb.tile([RPC, N], dt)
            nc.scalar.activation(diffsq[:], diff[:], Act.Square,
                                 bias=cEPS[0:RPC, :], scale=1.0)
            scr1 = sb.tile([RPC, N], dt)
            nc.vector.scalar_tensor_tensor(
                scr1[:], work[:], m2[:, 7:8], diffsq[:],
                op0=Alu.is_ge, op1=Alu.mult, accum_out=outsb[:, 0:1])

            nc.sync.dma_start(out_d[:, :], outsb[:])

    nc.compile()
    return nc


def _in_maps(yi, yi_t):
    yi = np.ascontiguousarray(np.asarray(yi, np.float32))
    yi_t = np.ascontiguousarray(np.asarray(yi_t, np.float32))
    yiT = yi.T
    maps = []
    for c in range(NCORES):
        r0 = c * RPC
        yiTp = np.roll(yiT, -r0, axis=1)
        pA = np.empty((128, 640), np.float32)
        pA[:, 0:32] = yi[r0:r0 + RPC, 0:128].T
        pA[:, 32:64] = yi[r0:r0 + RPC, 128:256].T
        pA[:, 64:96] = yi_t[r0:r0 + RPC, 0:128].T
        pA[:, 96:128] = yi_t[r0:r0 + RPC, 128:256].T
        pA[:, 128:384] = yiTp[0:128, :]
        pA[:, 384:640] = yiTp[128:256, :]
        pB = np.hstack([yi[r0:r0 + RPC], yi_t[r0:r0 + RPC]])
        maps.append({"pA": pA, "pB": np.ascontiguousarray(pB)})
    return maps


def kernel(yi, yi_t):
    from concourse.bass_utils import run_bass_kernel_spmd

    if "nc" not in _CACHE:
        _CACHE["nc"] = _build()
    nc = _CACHE["nc"]
    res = run_bass_kernel_spmd(nc, _in_maps(yi, yi_t), list(range(NCORES)))
    total = np.float64(0.0)
    for c in range(NCORES):
        total += np.sum(res.results[c]["out"], dtype=np.float64)
    total -= np.float64(N * K * T_THRESH)
    return np.float32(total)


# revision 4
# speedup vs baseline: 1.6132x; 1.1336x over previous
"""Trainium2 Bass kernel for nn_BLCD_Loss (retrieval_knn).

Math: for l2-normalized rows, ||a-b||^2 = 2 - 2*a.b, so all pairwise
distances come from small Gram matmuls per core (bf16 inputs, fp32 PSUM
accumulate, 1 cycle/row on the PE). Per-core inputs are column-rolled so
every core's self-pair lands on the local diagonal; self-exclusion is a
core-uniform -BIG*I accumulated into the Gram via a third matmul whose
operands are generated on-device (affine_select). Column norms come from
a replicated ones-matmul (all 32 partitions get the column sums), so no
broadcast matmul or PSUM copy is needed. Top-16 selection is two rounds
of DVE max8 + match_replace on bf16; the masked sum of (dis - dis_t)^2
is one fused scalar_tensor_tensor with accumulate. Exact fp32 is kept
where it matters (local row norms, cross dots, thresholded compare).

Sharding: 256 anchor rows -> 32 rows on each of 8 cores; each core
returns [32,2] partial rows (e1, e2 terms); the host sums them.
"""

import numpy as np

N, D, K = 256, 256, 16
M_MARGIN, T_THRESH, EPS = 0.6, 0.0025, 1e-12
NCORES, RPC = 8, 32
BIG = 1.0e5

_CACHE = {}


def _build():
    from concourse import bacc, mybir, tile
    import concourse.bass as bass

    dt = mybir.dt.float32
    bf = mybir.dt.bfloat16
    Alu = mybir.AluOpType
    Act = mybir.ActivationFunctionType

    nc = bacc.Bacc("TRN2", target_bir_lowering=False, debug=False)

    # pA cols (bf16): 0:32 yiLT0 | 32:64 yiLT1 | 64:96 yitT0 | 96:128 yitT1 |
    #                 128:384 yiT rows 0:128 (rolled) | 384:640 rows 128:256
    pA_d = nc.dram_tensor("pA", [128, 640], bf, kind="ExternalInput")
    pB_d = nc.dram_tensor("pB", [RPC, 2 * D], dt, kind="ExternalInput")
    out_d = nc.dram_tensor("out", [RPC, 2], dt, kind="ExternalOutput")

    with tile.TileContext(nc) as tc:
        with (
            tc.tile_pool(name="sb", bufs=1) as sb,
            tc.tile_pool(name="ps", bufs=1, space=bass.MemorySpace.PSUM) as ps,
        ):
            sbA = sb.tile([128, 640], bf)
            sbB = sb.tile([RPC, 2 * D], dt)

            # ---- Pool: small constants + on-device -BIG*I operands
            cEPS = sb.tile([128, 1], dt)
            nc.gpsimd.memset(cEPS[:], EPS)
            cHALF = sb.tile([128, 1], dt)
            nc.gpsimd.memset(cHALF[:], 0.5)
            zz = sb.tile([RPC, N], bf)
            nc.gpsimd.memset(zz[:], 0.0)
            eyeN = sb.tile([RPC, N], bf)
            nc.gpsimd.affine_select(
                eyeN[:], zz[:], [[1, N]], Alu.not_equal, -BIG,
                base=0, channel_multiplier=-1)
            i32 = sb.tile([RPC, RPC], bf)
            nc.gpsimd.affine_select(
                i32[:], zz[:, 0:RPC], [[1, RPC]], Alu.not_equal, 1.0,
                base=0, channel_multiplier=-1)

            # ---- input DMAs (SP queue)
            nc.sync.dma_start(sbA[:], pA_d[:, :])
            nc.sync.dma_start(sbB[:], pB_d[:, :])

            yiT0 = sbA[:, 128:384]
            yiT1 = sbA[:, 384:640]
            yiLT0 = sbA[:, 0:32]
            yiLT1 = sbA[:, 32:64]
            yitT0 = sbA[:, 64:96]
            yitT1 = sbA[:, 96:128]

            # ---- Act: dummy sqrt first so the act table loads at t~0
            dummy = sb.tile([1, 1], dt)
            nc.scalar.activation(dummy[:], cEPS[0:1, :], Act.Sqrt,
                                 bias=cEPS[0:1, :], scale=1.0)

            # ---- DVE: ones + squared yiT halves (bf16, fast)
            ones = sb.tile([128, RPC], bf)
            nc.vector.memset(ones[:], 1.0)
            sqA0 = sb.tile([128, D], bf)
            nc.vector.tensor_tensor(sqA0[:], yiT0, yiT0, op=Alu.mult)
            sqA1 = sb.tile([128, D], bf)
            nc.vector.tensor_tensor(sqA1[:], yiT1, yiT1, op=Alu.mult)

            # ---- Gram matmuls (bf16 = 1 cycle/row)
            ps_R = ps.tile([RPC, N], dt)
            nc.tensor.matmul(ps_R[:], yiLT0, yiT0, start=True, stop=False)
            nc.tensor.matmul(ps_R[:], yiLT1, yiT1, start=False, stop=False)
            nc.tensor.matmul(ps_R[:], i32[:], eyeN[:], start=False, stop=True)
            ps_s = ps.tile([RPC, N], dt)
            nc.tensor.matmul(ps_s[:], ones[:], sqA0[:], start=True, stop=False)
            nc.tensor.matmul(ps_s[:], ones[:], sqA1[:], start=False, stop=True)
            ps_Rt = ps.tile([RPC, N], dt)
            nc.tensor.matmul(ps_Rt[:], yitT0, yiT0, start=True, stop=False)
            nc.tensor.matmul(ps_Rt[:], yitT1, yiT1, start=False, stop=True)

            # ---- local row norms + cross dot from the B tile (fp32 exact)
            sqB = sb.tile([RPC, 2 * D], dt)
            nc.vector.tensor_tensor(sqB[:], sbB[:], sbB[:], op=Alu.mult)
            nrm2 = sb.tile([RPC, 2], dt)
            nc.vector.tensor_reduce(
                nrm2[:], sqB[:].rearrange("p (g x) -> p g x", g=2),
                axis=mybir.AxisListType.X, op=Alu.add)
            scr0 = sb.tile([RPC, N], dt)
            dot_ii = sb.tile([RPC, 1], dt)
            nc.vector.tensor_tensor_reduce(
                scr0[:], sbB[:, 0:256], sbB[:, 256:512], 1.0, 0.0,
                op0=Alu.mult, op1=Alu.add, accum_out=dot_ii[:])

            # ---- t_b = sqrt(colsum + eps); inv_b = 1/t_b
            t_b = sb.tile([RPC, N], dt)
            nc.scalar.activation(t_b[:], ps_s[:], Act.Sqrt,
                                 bias=cEPS[0:RPC, :], scale=1.0)
            t2 = sb.tile([RPC, 2], dt)
            nc.scalar.activation(t2[:], nrm2[:], Act.Sqrt,
                                 bias=cEPS[0:RPC, :], scale=1.0)
            inv_b = sb.tile([RPC, N], dt)
            nc.vector.reciprocal(inv_b[:], t_b[:])
            inv2 = sb.tile([RPC, 2], dt)
            nc.vector.reciprocal(inv2[:], t2[:])
            sc2 = sb.tile([RPC, 2], dt)
            nc.vector.tensor_scalar_mul(sc2[:], inv2[:], -0.5)
            u = sb.tile([RPC, 1], dt)
            nc.vector.tensor_tensor(u[:], dot_ii[:], inv2[:, 0:1], op=Alu.mult)

            # ---- normalized Grams (bf16 outputs for the fast top-k chain)
            work = sb.tile([RPC, N], bf)
            nc.vector.tensor_tensor(work[:], ps_R[:], inv_b[:], op=Alu.mult)
            # H1 on Pool so DVE can start the max chain immediately
            H1 = sb.tile([RPC, N], bf)
            nc.gpsimd.tensor_tensor(H1[:], ps_Rt[:], inv_b[:], op=Alu.mult)

            # ---- top-16 threshold per row (self sits at -BIG on the diag)
            m1 = sb.tile([RPC, 8], bf)
            nc.vector.max(out=m1[:], in_=work[:])
            w2 = sb.tile([RPC, N], bf)
            nc.vector.match_replace(
                out=w2[:], in_to_replace=m1[:], in_values=work[:],
                imm_value=-BIG)
            m2 = sb.tile([RPC, 8], bf)
            nc.vector.max(out=m2[:], in_=w2[:])

            # ---- distances (row scale folded into activation scale)
            dis = sb.tile([RPC, N], bf)
            nc.scalar.activation(dis[:], work[:], Act.Sqrt,
                                 bias=cHALF[0:RPC, :], scale=sc2[:, 0:1])
            dis_t = sb.tile([RPC, N], bf)
            nc.scalar.activation(dis_t[:], H1[:], Act.Sqrt,
                                 bias=cHALF[0:RPC, :], scale=sc2[:, 1:2])
            dis2 = sb.tile([RPC, 1], dt)
            nc.scalar.activation(dis2[:], m1[:, 0:1], Act.Sqrt,
                                 bias=cHALF[0:RPC, :], scale=sc2[:, 0:1])
            dis_ii = sb.tile([RPC, 1], dt)
            nc.scalar.activation(dis_ii[:], u[:], Act.Sqrt,
                                 bias=cHALF[0:RPC, :], scale=sc2[:, 1:2])

            outsb = sb.tile([RPC, 2], dt)

            # ---- e2 = relu(dis_ii + (margin - dis2))
            bias2 = sb.tile([RPC, 1], dt)
            nc.vector.tensor_scalar(
                bias2[:], dis2[:], -1.0, M_MARGIN, op0=Alu.mult, op1=Alu.add)
            nc.scalar.activation(outsb[:, 1:2], dis_ii[:], Act.Relu,
                                 bias=bias2[:], scale=1.0)

            # ---- e1 = sum over neighbors of (dis - dis_t)^2, fused
            diff = sb.tile([RPC, N], bf)
            nc.vector.tensor_sub(diff[:], dis[:], dis_t[:])
            diffsq = sb.tile([RPC, N], bf)
            nc.vector.tensor_tensor(diffsq[:], diff[:], diff[:], op=Alu.mult)
            scr1 = sb.tile([RPC, N], bf)
            nc.vector.scalar_tensor_tensor(
                scr1[:], work[:], m2[:, 7:8], diffsq[:],
                op0=Alu.is_ge, op1=Alu.mult, accum_out=outsb[:, 0:1])

            nc.sync.dma_start(out_d[:, :], outsb[:])

    nc.compile()
    return nc


def _in_maps(yi, yi_t):
    import ml_dtypes
    bf16 = ml_dtypes.bfloat16
    yi = np.ascontiguousarray(np.asarray(yi, np.float32))
    yi_t = np.ascontiguousarray(np.asarray(yi_t, np.float32))
    yiT = yi.T
    maps = []
    for c in range(NCORES):
        r0 = c * RPC
        yiTp = np.roll(yiT, -r0, axis=1)
        pA = np.empty((128, 640), np.float32)
        pA[:, 0:32] = yi[r0:r0 + RPC, 0:128].T
        pA[:, 32:64] = yi[r0:r0 + RPC, 128:256].T
        pA[:, 64:96] = yi_t[r0:r0 + RPC, 0:128].T
        pA[:, 96:128] = yi_t[r0:r0 + RPC, 128:256].T
        pA[:, 128:384] = yiTp[0:128, :]
        pA[:, 384:640] = yiTp[128:256, :]
        pB = np.hstack([yi[r0:r0 + RPC], yi_t[r0:r0 + RPC]])
        maps.append({"pA": pA.astype(bf16), "pB": np.ascontiguousarray(pB)})
    return maps


def kernel(yi, yi_t):
    from concourse.bass_utils import run_bass_kernel_spmd

    if "nc" not in _CACHE:
        _CACHE["nc"] = _build()
    nc = _CACHE["nc"]
    res = run_bass_kernel_spmd(nc, _in_maps(yi, yi_t), list(range(NCORES)))
    total = np.float64(0.0)
    for c in range(NCORES):
        total += np.sum(res.results[c]["out"], dtype=np.float64)
    total -= np.float64(N * K * T_THRESH)
    return np.float32(total)
